# revision 1
# baseline (speedup 1.0000x reference)
"""DeepWelchTransform kernel for Trainium2 (8 NeuronCores).

Math
----
The reference computes, per batch row b (B=1024, S=16384, NPERSEG=1024,
STEP=256, NWIN=61):

    fr[b] = mean_w  sum_t input[b, 256*w + t] *  cos(2*pi*freqs[t])
    fi[b] = mean_w  sum_t input[b, 256*w + t] * (-sin(2*pi*freqs[t]))
    out[b] = (fr[b]^2 + fi[b]^2) * fc_w + fc_b

Everything up to the square is linear in `input`, so the window
gather + per-window dot + mean folds into a single length-S dot product
per batch row with "effective" weight vectors

    c_eff[s] = (1/61) * sum_{w : 0 <= s-256w < 1024} cos(ang[s-256w])
    s_eff[s] = (1/61) * sum_{w : 0 <= s-256w < 1024} -sin(ang[s-256w])

(the host folds these from `freqs` in float64 — O(S) work). The device
work is then two matvecs [1024, 16384] @ [16384] → purely HBM-bound
(64 MiB input read; ~23 us/core at the ~358 GB/s per-core HBM limit).

Sharding
--------
The sequence dim is split across the 8 cores (2048 s-positions each);
every core sees all 1024 batch rows and produces partial (fr, fi) pairs
for all rows. No on-device communication: the host sums the 8 partials
and applies the final square + affine (a few KFLOP on [1024]).

Per-core device kernel
----------------------
The 2048 s-positions map to 128 SBUF partitions x 16 columns
(s_local = p*16 + rs). For each rs, TensorE matmuls contract over the
128 partitions: stationary = [128, M] weight slices, moving = [128, 512]
batch slabs, accumulated over all 16 rs into PSUM. The input shard is
pre-arranged on the host to [p][rs][b] so the DMA is perfectly
sequential (64 KiB/partition).

Precision ("bf8p" default): x is split hi/lo as bf16 + scaled-fp8
residual (3 B/elem of DMA — 25% below the fp32 roofline's traffic).
The bf16 stationary packs (wh_c, wh_s, wl_c, wl_s) into M=4 columns so
one xh pass yields main + w-correction products simultaneously; the fp8
residual stream multiplies a scaled fp8 stationary (M=4 with a
second-order w correction) into its own PSUM group. The host unscales
and sums the PSUM rows. Measured max relative error vs the fp32
reference: ~2.8e-4 (scale-relative absmax ~3.5e-5). The "bf16p" variant
(4 B/elem, ~2.3e-5 max rel err) is one flag away.

PE warm-up junk matmuls run during the first chunk's DMA so the HAM
clock gate releases before real work; a small final DMA chunk keeps the
post-DMA matmul tail short. Modeled single-shot: ~25 us/core; steady
state is HBM-bound at ~6 MiB / core read.
"""

import numpy as np

import concourse.bass as bass
import concourse.tile as tile
from concourse import bacc, mybir
from concourse.bass_utils import run_bass_kernel_spmd

N_CORES = 8
B, S = 1024, 16384
NPERSEG, STEP = 1024, 256
NWIN = (S - NPERSEG) // STEP + 1  # 61
S_PC = S // N_CORES  # 2048 s-positions per core
P = 128  # SBUF partitions
RS = S_PC // P  # 16 s-columns per partition
N_HALF = 512  # moving free size (1024 batch cols / 2)
RS_PER_CHUNK = 2  # DMA chunk granularity (2 rs cols: 0.5 MiB per hi/lo DMA)
# full-size chunks + single-rs tail chunks (shorter post-DMA matmul tail)
N_CHUNKS = RS // RS_PER_CHUNK - 1 + RS_PER_CHUNK

_f32 = mybir.dt.float32
_f32r = mybir.dt.float32r
_bf16 = mybir.dt.bfloat16
_f16 = mybir.dt.float16
_f8 = mybir.dt.float8e4

# The fp8 residual stream (bf8p) pre-scales xl / wb / wb2 on the host with
# adaptive power-of-2 factors (chosen per call from the data's max-abs so
# e4m3 never saturates); the host divides the stream-B partials back down.

_NC_CACHE = {}


# Per-precision stream configs.
#   x: list of (name, dtype) moving tensors
#   w: list of (name, dtype, m) stationary tensors (m = packed column count)
#   streams: (x_idx, w_idx, group) matmul products; each group accumulates
#            into its own PSUM bank pair and emits its own [m, B] output.
_CONFIGS = {
    # bf16 hi/lo split of x AND w, with the w hi/lo (x cos/sin) packed into
    # the stationary's M columns: stationary [128, 4] = (whc, whs, wlc, wls).
    # Two moving passes (xh, xl) against the same stationary accumulate the
    # full (xh+xl)*(wh+wl) split across PSUM rows {0,2} (cos) / {1,3} (sin);
    # the host sums even/odd rows.  PE cost: 2 passes, M is free.
    "bf16p": {
        "x": [("xh", _bf16), ("xl", _bf16)],
        "w": [("w4", _bf16, 4)],
        "streams": [(0, 0, 0), (1, 0, 0)],
    },
    # like bf16p but the xl residual is shipped as scaled fp8 (3 B/elem of
    # DMA instead of 4) and multiplies a scaled fp8 copy of w in its own
    # PSUM group. The fp8 stationary carries 4 columns: (wb_c, wb_s) plus a
    # second-order correction pair (wb2_c, wb2_s) encoding the fp8
    # quantization error of wb — M-packing makes the correction free. The
    # host unscales group-1 rows {0,1} by 1/(XL_SCALE*WB_SCALE) and rows
    # {2,3} by 1/(XL_SCALE*WB2_SCALE).
    "bf8p": {
        "x": [("xh", _bf16), ("xl", _f8)],
        "w": [("w4", _bf16, 4), ("wb", _f8, 4)],
        "streams": [(0, 0, 0), (1, 1, 1)],
    },
    "f32r": {
        "x": [("x", _f32r)],
        "w": [("w", _f32r, 2)],
        "streams": [(0, 0, 0)],
    },
    "f16x2": {
        "x": [("xh", _f16), ("xl", _f16)],
        "w": [("wh", _f16, 2)],
        "streams": [(0, 0, 0), (1, 0, 0)],
    },
    "bf16x3": {
        "x": [("xh", _bf16), ("xl", _bf16)],
        "w": [("wh", _bf16, 2), ("wl", _bf16, 2)],
        "streams": [(0, 0, 0), (0, 1, 0), (1, 0, 0)],
    },
}


def _build_bass(repeat=1, precision="bf8p"):
    cfg = _CONFIGS[precision]
    x_specs, w_specs, streams = cfg["x"], cfg["w"], cfg["streams"]
    groups = sorted({g for _, _, g in streams})
    grp_m = {g: max(w_specs[wi][2] for _, wi, gg in streams if gg == g) for g in groups}
    m_max = max(grp_m.values())

    nc = bacc.Bacc("TRN2", debug=False)
    # one output tensor: group g occupies columns [g*B, (g+1)*B) — both
    # PSUM groups sit on partitions 0..m-1, so a single SBUF staging tile
    # and a single out-DMA cover all of them
    o_d = nc.dram_tensor(
        "o", [m_max, len(groups) * B], _f32, kind="ExternalOutput"
    ).ap()
    x_ds = [
        nc.dram_tensor(n, [P, RS * B], dt, kind="ExternalInput").ap()
        for n, dt in x_specs
    ]
    w_ds = [
        nc.dram_tensor(n, [P, RS * m], dt, kind="ExternalInput").ap()
        for n, dt, m in w_specs
    ]

    with tile.TileContext(nc) as tc:
        with (
            tc.tile_pool(name="xp", bufs=N_CHUNKS) as xp,
            tc.tile_pool(name="wp", bufs=1) as wp,
            tc.tile_pool(name="pp", bufs=1, space="PSUM") as pp,
            tc.tile_pool(name="op", bufs=2) as op,
        ):
            # weights go via SWDGE (gpsimd) so the x chunk DMAs own the
            # HWDGE ring from t=0
            w_sbs = []
            for i, (n, dt, m) in enumerate(w_specs):
                w_sb = wp.tile([P, RS * m], dt, name=f"w_sb{i}", tag=f"w{i}")
                nc.gpsimd.dma_start(w_sb[:], w_ds[i][:])
                w_sbs.append(w_sb)

            # PE warm-up: junk matmuls on a zeroed tile keep the PE busy
            # during the first chunk's DMA so the HAM clock-gate releases
            # (1.2 -> 2.4 GHz) before the real matmuls start. The final
            # junk matmuls read the w_sb tiles so the first real matmul
            # carries a single sync-wait (the fused LDW+MM pair has few
            # wait slots).
            junk = wp.tile([P, N_HALF], x_specs[0][1], name="junk", tag="junk")
            nc.vector.memset(junk[:], 0.0)
            scratch = pp.tile(
                [max(grp_m.values()), N_HALF],
                _f32,
                name="scratch",
                tag="scratch",
                bufs=1,
            )
            for _ in range(8):
                nc.tensor.matmul(
                    scratch[:2, :], junk[:, 0:2], junk[:], start=True, stop=True
                )
            for i, w_sb in enumerate(w_sbs):
                m = w_specs[i][2]
                nc.tensor.matmul(
                    scratch[:m, 0 : RS * m],
                    w_sb[:, 0:m],
                    w_sb[:],
                    start=True,
                    stop=True,
                )

            # chunk sizes in rs columns; a small final chunk shortens the
            # post-DMA matmul tail
            chunk_rs = [RS_PER_CHUNK] * (RS // RS_PER_CHUNK - 1) + [1] * (
                RS_PER_CHUNK
            )
            assert sum(chunk_rs) == RS

            first_s = {
                g: next(s for s in streams if s[2] == g) for g in groups
            }
            last_s = {
                g: next(s for s in reversed(streams) if s[2] == g)
                for g in groups
            }

            for it in range(repeat):
                ps = {
                    (g, h): pp.tile(
                        [grp_m[g], N_HALF],
                        _f32,
                        name=f"ps{g}_{h}_{it}",
                        tag=f"ps{g}_{h}",
                    )
                    for g in groups
                    for h in range(2)
                }
                rs0 = 0
                for c, crs in enumerate(chunk_rs):
                    x_cs = []
                    for i, (n, dt) in enumerate(x_specs):
                        x_c = xp.tile(
                            [P, crs * B], dt, name=f"x{i}_{it}_{c}", tag=f"x{i}"
                        )
                        nc.sync.dma_start(
                            x_c[:], x_ds[i][:, rs0 * B : (rs0 + crs) * B]
                        )
                        x_cs.append(x_c)
                    for r in range(crs):
                        rs = rs0 + r
                        last_rs = rs == RS - 1
                        # streams stay in xh-first order: on the final column
                        # the xh-dependent matmuls start as soon as xh lands
                        # (xl arrives last), and each bank's copy launches
                        # right after its own final matmul
                        s_order = streams
                        for s in s_order:
                            xi, wi, g = s
                            m = w_specs[wi][2]
                            lhsT = w_sbs[wi][:, m * rs : m * (rs + 1)]
                            # on the final column, finish bank 1 first so its
                            # PSUM->SBUF copy overlaps bank 0's last matmuls
                            h_order = (1, 0) if last_rs else (0, 1)
                            for h in h_order:
                                rhs = x_cs[xi][
                                    :, r * B + h * N_HALF : r * B + (h + 1) * N_HALF
                                ]
                                nc.tensor.matmul(
                                    ps[(g, h)][:],
                                    lhsT,
                                    rhs,
                                    start=(rs == 0 and s == first_s[g]),
                                    stop=(last_rs and s == last_s[g]),
                                )
                    rs0 += crs

                out_sb = op.tile(
                    [m_max, len(groups) * B],
                    _f32,
                    name=f"out_sb_{it}",
                    tag="out_sb",
                )
                for g in groups:
                    m = grp_m[g]
                    nc.vector.tensor_copy(
                        out_sb[:m, g * B + N_HALF : (g + 1) * B], ps[(g, 1)][:]
                    )
                    nc.scalar.copy(
                        out_sb[:m, g * B : g * B + N_HALF], ps[(g, 0)][:]
                    )
                nc.sync.dma_start(o_d[:], out_sb[:])
    nc.compile()
    return nc


def _get_nc(repeat=1, precision="bf8p"):
    key = (repeat, precision)
    if key not in _NC_CACHE:
        _NC_CACHE[key] = _build_bass(repeat, precision)
    return _NC_CACHE[key]


def _fold_weights(freqs):
    """Fold freqs -> effective per-position cos/sin weights [S, 2] (f32)."""
    ang = 2.0 * np.pi * np.asarray(freqs, dtype=np.float64)
    cosv = np.cos(ang)
    msinv = -np.sin(ang)
    c_eff = np.zeros(S, np.float64)
    s_eff = np.zeros(S, np.float64)
    for w in range(NWIN):
        c_eff[w * STEP : w * STEP + NPERSEG] += cosv
        s_eff[w * STEP : w * STEP + NPERSEG] += msinv
    c_eff /= NWIN
    s_eff /= NWIN
    return np.stack([c_eff, s_eff], axis=-1).astype(np.float32)  # [S, 2]


def _pow2_scale(max_abs, target=120.0):
    """Largest power-of-2 scale keeping max_abs*scale <= target.

    ml_dtypes.float8_e4m3 (IEEE, used for mybir float8e4) has max finite
    240 and overflows to inf — stay at half that."""
    if max_abs <= 0 or not np.isfinite(max_abs):
        return 1.0
    return float(2.0 ** np.floor(np.log2(target / max_abs)))


def _run(input, freqs, fc_w, fc_b, trace=False, precision="bf8p"):
    input = np.ascontiguousarray(np.asarray(input, dtype=np.float32))
    eff = _fold_weights(freqs)

    # rearrange to the device layout x[p, rs*B + b] = shard[b, p*RS + rs]
    x_dev = np.ascontiguousarray(
        input.reshape(B, N_CORES, P, RS).transpose(1, 2, 3, 0)
    )  # [N_CORES, P, RS, B]
    w_dev = eff.reshape(N_CORES, P, RS * 2)

    # adaptive (host-side only) fp8 scales: the device multiplies scaled
    # values, the host divides the partials back down
    scales = {}
    if precision == "bf8p":
        import ml_dtypes

        f8_np = mybir.dt.np(_f8)
        xl_all = input - input.astype(ml_dtypes.bfloat16).astype(np.float32)
        scales["xl"] = _pow2_scale(np.abs(xl_all).max())
        scales["wb"] = _pow2_scale(np.abs(eff).max())
        werr_all = eff - (eff * scales["wb"]).astype(f8_np).astype(
            np.float32
        ) / scales["wb"]
        scales["wb2"] = _pow2_scale(np.abs(werr_all).max())
        del xl_all, werr_all

    in_maps = []
    for k in range(N_CORES):
        x_host = x_dev[k].reshape(P, RS * B)
        w_host = w_dev[k]
        if precision in ("bf16p", "bf8p"):
            import ml_dtypes

            xh = x_host.astype(ml_dtypes.bfloat16)
            xl_f32 = x_host - xh.astype(np.float32)
            w2 = w_host.reshape(P, RS, 2)
            wh = w2.astype(ml_dtypes.bfloat16)
            wl = (w2 - wh.astype(np.float32)).astype(ml_dtypes.bfloat16)
            w4 = np.concatenate([wh, wl], axis=-1).reshape(P, RS * 4)
            m = {"xh": xh, "w4": np.ascontiguousarray(w4)}
            if precision == "bf16p":
                m["xl"] = np.ascontiguousarray(xl_f32.astype(ml_dtypes.bfloat16))
            else:
                f8 = mybir.dt.np(_f8)
                xl_s, wb_s, wb2_s = scales["xl"], scales["wb"], scales["wb2"]
                m["xl"] = np.ascontiguousarray((xl_f32 * xl_s).astype(f8))
                wb = (w2 * wb_s).astype(f8)
                werr = w2 - wb.astype(np.float32) / wb_s
                wb2 = (werr * wb2_s).astype(f8)
                m["wb"] = np.ascontiguousarray(
                    np.concatenate([wb, wb2], axis=-1).reshape(P, RS * 4)
                )
            in_maps.append(m)
        elif precision == "f32r":
            in_maps.append({"x": x_host, "w": np.ascontiguousarray(w_host)})
        elif precision == "f16x2":
            xh = x_host.astype(np.float16)
            xl = (x_host - xh.astype(np.float32)).astype(np.float16)
            wh = np.ascontiguousarray(w_host).astype(np.float16)
            in_maps.append({"xh": xh, "xl": np.ascontiguousarray(xl), "wh": wh})
        else:
            import ml_dtypes

            xh = x_host.astype(ml_dtypes.bfloat16)
            xl = (x_host - xh.astype(np.float32)).astype(ml_dtypes.bfloat16)
            wh = w_host.astype(ml_dtypes.bfloat16)
            wl = (w_host - wh.astype(np.float32)).astype(ml_dtypes.bfloat16)
            in_maps.append(
                {
                    "xh": xh,
                    "xl": np.ascontiguousarray(xl),
                    "wh": np.ascontiguousarray(wh),
                    "wl": np.ascontiguousarray(wl),
                }
            )

    last_exc = None
    for attempt in range(3):
        try:
            res = run_bass_kernel_spmd(
                _get_nc(1, precision),
                in_maps,
                core_ids=list(range(N_CORES)),
                trace=trace,
            )
            break
        except Exception as e:  # transient NRT/device hiccups: retry
            last_exc = e
            import time as _time

            _time.sleep(2.0)
    else:
        raise last_exc

    fr = np.zeros(B, np.float64)
    fi = np.zeros(B, np.float64)
    for r in res.results:
        o = r["o"]
        g0 = o[:, 0:B]
        fr += g0[0::2].sum(axis=0, dtype=np.float64)
        fi += g0[1::2].sum(axis=0, dtype=np.float64)
        if o.shape[1] > B:  # fp8 residual group (bf8p)
            g1 = o[:, B : 2 * B]
            s1 = scales["xl"] * scales["wb"]
            s2 = scales["xl"] * scales["wb2"]
            fr += g1[0].astype(np.float64) / s1
            fi += g1[1].astype(np.float64) / s1
            fr += g1[2].astype(np.float64) / s2
            fi += g1[3].astype(np.float64) / s2
    psd = fr**2 + fi**2
    out = psd * float(np.asarray(fc_w).reshape(-1)[0]) + float(
        np.asarray(fc_b).reshape(-1)[0]
    )
    return out.astype(np.float32).reshape(B, 1), res


def kernel(input, freqs, fc_w, fc_b):
    out, _ = _run(input, freqs, fc_w, fc_b, trace=False)
    return out



# revision 5
# speedup vs baseline: 1.3322x; 1.3322x over previous
"""DeepWelchTransform kernel for Trainium2 (8 NeuronCores).

Math
----
The reference computes, per batch row b (B=1024, S=16384, NPERSEG=1024,
STEP=256, NWIN=61):

    fr[b] = mean_w  sum_t input[b, 256*w + t] *  cos(2*pi*freqs[t])
    fi[b] = mean_w  sum_t input[b, 256*w + t] * (-sin(2*pi*freqs[t]))
    out[b] = (fr[b]^2 + fi[b]^2) * fc_w + fc_b

Everything up to the square is linear in `input`, so the window
gather + per-window dot + mean folds into a single length-S dot product
per batch row with "effective" weight vectors

    c_eff[s] = (1/61) * sum_{w : 0 <= s-256w < 1024} cos(ang[s-256w])
    s_eff[s] = (1/61) * sum_{w : 0 <= s-256w < 1024} -sin(ang[s-256w])

(the host folds these from `freqs` in float64 — O(S) work). The device
work is then two matvecs [1024, 16384] @ [16384] → purely HBM-bound.

Sharding
--------
The sequence dim is split across the 8 cores (2048 s-positions each);
every core sees all 1024 batch rows and produces partial (fr, fi) pairs
for all rows. No on-device communication: the host sums the 8 partials
and applies the final square + affine (a few KFLOP on [1024]).

Per-core device kernel ("f8dr")
-------------------------------
x is shipped as TWO fp8(e4m3) planes sharing a single power-of-2 scale:
hi = e4m3(x*xs), lo = e4m3(x*xs - hi) — 2 B/elem of DMA (4 MiB/core,
the byte floor for this error budget; measured max rel err ~4.7e-3 vs
the 2e-2 gate). The hi/lo planes ride the TensorE DoubleRow perf mode
as the two k-tiles of each fp8 matmul: moving [128, 2, 512], stationary
[128, 2, M] with the weight columns duplicated across both k-planes, so
one instruction computes w.(hi+lo) at 0.5 cycles/row.

The stationary packs M=6 columns: (wb_c, wb_s) = e4m3(w*ws) plus two
correction pairs (wb2, wb3) encoding the successive fp8 quantization
residuals of w — M-packing makes both corrections free on the PE. The
host unscales rows {0,1}/{2,3}/{4,5} by 1/(xs*ws{1,2,3}) and sums.

The 2048 s-positions map to 128 partitions x 16 columns
(s_local = p*16 + rs); for each rs TensorE contracts the 128 partitions
x 2 k-planes into PSUM over all 16 rs. The input shard is pre-arranged
on the host to [p][rs][kt][b] so the DMA is perfectly sequential
(32 KiB/partition). Chunked DMA (4 KiB/partition chunks, small tail
chunks) keeps TensorE fed and the post-DMA matmul tail short.
"""

import numpy as np

import concourse.bass as bass
import concourse.tile as tile
from concourse import bacc, mybir
from concourse.bass_utils import run_bass_kernel_spmd

N_CORES = 8
B, S = 1024, 16384
NPERSEG, STEP = 1024, 256
NWIN = (S - NPERSEG) // STEP + 1  # 61
S_PC = S // N_CORES  # 2048 s-positions per core
P = 128  # SBUF partitions
RS = S_PC // P  # 16 s-columns per partition
KT = 2  # DoubleRow k-tiles: (hi, lo) fp8 planes
M = 16  # stationary columns (dual-fp8 LDW needs >=16); first M_OUT are real
M_OUT = 6  # (wb_c, wb_s, wb2_c, wb2_s, wb3_c, wb3_s); rest zero-padded
N_HALF = 512  # moving free size per matmul (1024 batch cols / 2)
RS_PER_CHUNK = 2  # DMA chunk granularity
# full-size chunks + single-rs tail chunks (shorter post-DMA matmul tail)
N_CHUNKS = RS // RS_PER_CHUNK - 1 + RS_PER_CHUNK

_f32 = mybir.dt.float32
_f8 = mybir.dt.float8e4

_NC_CACHE = {}


def _build_bass(repeat=1):
    nc = bacc.Bacc("TRN2", debug=False)
    o_d = nc.dram_tensor("o", [M_OUT, KT * N_HALF], _f32, kind="ExternalOutput").ap()
    x_d = nc.dram_tensor("x", [P, RS * KT, B], _f8, kind="ExternalInput").ap()
    w_d = nc.dram_tensor("w", [P, RS * KT, M], _f8, kind="ExternalInput").ap()

    with tile.TileContext(nc) as tc:
        with (
            tc.tile_pool(name="xp", bufs=N_CHUNKS) as xp,
            tc.tile_pool(name="wp", bufs=1) as wp,
            tc.tile_pool(name="pp", bufs=1, space="PSUM") as pp,
            tc.tile_pool(name="op", bufs=2) as op,
        ):
            # weights go via SWDGE (gpsimd) so the x chunk DMAs own the
            # HWDGE ring from t=0
            w_sb = wp.tile([P, RS * KT, M], _f8, name="w_sb", tag="w")
            nc.gpsimd.dma_start(w_sb[:], w_d[:])

            # PE warm-up: junk matmuls on a zeroed tile keep the PE busy
            # during the first chunk's DMA so the clock gate releases
            # before the real matmuls start. The final junk matmul reads
            # w_sb so the first real matmul only waits on its x chunk.
            junk = wp.tile([P, KT, N_HALF], _f8, name="junk", tag="junk")
            nc.vector.memset(junk[:], 0.0)
            scratch = pp.tile([M, N_HALF], _f32, name="scratch", tag="scratch", bufs=1)
            for _ in range(6):
                nc.tensor.matmul(
                    scratch[:2, :],
                    junk[:, 0:1, 0:2],
                    junk[:, 0:1, :],
                    start=True,
                    stop=True,
                )
            for _ in range(2):
                nc.tensor.matmul(
                    scratch[:, :],
                    w_sb[:, 0:KT, :],
                    junk[:],
                    start=True,
                    stop=True,
                    perf_mode=mybir.MatmulPerfMode.DoubleRow,
                )

            # chunk sizes in rs columns; small final chunks shorten the
            # post-DMA matmul tail
            chunk_rs = [RS_PER_CHUNK] * (RS // RS_PER_CHUNK - 1) + [1] * RS_PER_CHUNK
            assert sum(chunk_rs) == RS

            for it in range(repeat):
                ps = {
                    h: pp.tile([M, N_HALF], _f32, name=f"ps{h}_{it}", tag=f"ps{h}")
                    for h in range(2)
                }
                rs0 = 0
                for c, crs in enumerate(chunk_rs):
                    x_c = xp.tile(
                        [P, crs * KT, B], _f8, name=f"x_{it}_{c}", tag="x"
                    )
                    nc.sync.dma_start(
                        x_c[:], x_d[:, rs0 * KT : (rs0 + crs) * KT, :]
                    )
                    for r in range(crs):
                        rs = rs0 + r
                        last_rs = rs == RS - 1
                        # on the final column, finish bank 1 first so its
                        # PSUM->SBUF copy overlaps bank 0's last matmul
                        h_order = (1, 0) if last_rs else (0, 1)
                        for h in h_order:
                            nc.tensor.matmul(
                                ps[h][:],
                                w_sb[:, rs * KT : (rs + 1) * KT, :],
                                x_c[:, r * KT : (r + 1) * KT, h * N_HALF : (h + 1) * N_HALF],
                                start=(rs == 0),
                                stop=last_rs,
                                perf_mode=mybir.MatmulPerfMode.DoubleRow,
                            )
                    rs0 += crs

                out_sb = op.tile(
                    [M_OUT, KT * N_HALF], _f32, name=f"out_sb_{it}", tag="out_sb"
                )
                nc.vector.tensor_copy(
                    out_sb[:, N_HALF : KT * N_HALF], ps[1][:M_OUT, :]
                )
                nc.scalar.copy(out_sb[:, 0:N_HALF], ps[0][:M_OUT, :])
                nc.sync.dma_start(o_d[:], out_sb[:])
    nc.compile()
    return nc


def _get_nc(repeat=1, precision="f8dr"):
    key = repeat
    if key not in _NC_CACHE:
        _NC_CACHE[key] = _build_bass(repeat)
    return _NC_CACHE[key]


def _fold_weights(freqs):
    """Fold freqs -> effective per-position cos/sin weights [S, 2] (f32)."""
    ang = 2.0 * np.pi * np.asarray(freqs, dtype=np.float64)
    cosv = np.cos(ang)
    msinv = -np.sin(ang)
    c_eff = np.zeros(S, np.float64)
    s_eff = np.zeros(S, np.float64)
    for w in range(NWIN):
        c_eff[w * STEP : w * STEP + NPERSEG] += cosv
        s_eff[w * STEP : w * STEP + NPERSEG] += msinv
    c_eff /= NWIN
    s_eff /= NWIN
    return np.stack([c_eff, s_eff], axis=-1).astype(np.float32)  # [S, 2]


def _pow2_scale(max_abs, target=120.0):
    """Largest power-of-2 scale keeping max_abs*scale <= target.

    ml_dtypes.float8_e4m3 (IEEE, used for mybir float8e4) has max finite
    240 and overflows to inf — stay at half that."""
    if max_abs <= 0 or not np.isfinite(max_abs):
        return 1.0
    return float(2.0 ** np.floor(np.log2(target / max_abs)))


def _run(input, freqs, fc_w, fc_b, trace=False, precision="f8dr"):
    f8_np = mybir.dt.np(_f8)
    input = np.ascontiguousarray(np.asarray(input, dtype=np.float32))
    eff = _fold_weights(freqs)  # [S, 2] f32

    # x -> single-scale fp8 hi/lo planes
    xs = _pow2_scale(np.abs(input).max())
    hi = (input * xs).astype(f8_np)
    lo = (input * xs - hi.astype(np.float32)).astype(f8_np)

    # device layout x[k][p][rs][kt][b];  s = k*2048 + p*16 + rs
    x_dev = np.empty((N_CORES, P, RS, KT, B), f8_np)
    x_dev[:, :, :, 0, :] = hi.reshape(B, N_CORES, P, RS).transpose(1, 2, 3, 0)
    x_dev[:, :, :, 1, :] = lo.reshape(B, N_CORES, P, RS).transpose(1, 2, 3, 0)
    x_dev = x_dev.reshape(N_CORES, P, RS * KT, B)

    # w -> fp8 wb + two successive fp8 residual corrections (wb2, wb3),
    # each with its own power-of-2 scale; columns (c, s) interleaved and
    # the whole M-pack duplicated across both DoubleRow k-planes
    ws1 = _pow2_scale(np.abs(eff).max())
    wb = (eff * ws1).astype(f8_np)
    r1 = eff - wb.astype(np.float32) / ws1
    ws2 = _pow2_scale(np.abs(r1).max())
    wb2 = (r1 * ws2).astype(f8_np)
    r2 = eff - wb.astype(np.float32) / ws1 - wb2.astype(np.float32) / ws2
    ws3 = _pow2_scale(np.abs(r2).max())
    wb3 = (r2 * ws3).astype(f8_np)
    w6 = np.concatenate(
        [wb, wb2, wb3, np.zeros((S, M - M_OUT), f8_np)], axis=-1
    )  # [S, M] (zero-padded to the dual-fp8 LDW minimum width)
    w_dev = np.empty((N_CORES, P, RS, KT, M), f8_np)
    w_dev[:, :, :, 0, :] = w6.reshape(N_CORES, P, RS, M)
    w_dev[:, :, :, 1, :] = w6.reshape(N_CORES, P, RS, M)
    w_dev = w_dev.reshape(N_CORES, P, RS * KT, M)

    in_maps = [
        {
            "x": np.ascontiguousarray(x_dev[k]),
            "w": np.ascontiguousarray(w_dev[k]),
        }
        for k in range(N_CORES)
    ]

    last_exc = None
    for attempt in range(3):
        try:
            res = run_bass_kernel_spmd(
                _get_nc(1),
                in_maps,
                core_ids=list(range(N_CORES)),
                trace=trace,
            )
            break
        except Exception as e:  # transient NRT/device hiccups: retry
            last_exc = e
            import time as _time

            _time.sleep(2.0)
    else:
        raise last_exc

    fr = np.zeros(B, np.float64)
    fi = np.zeros(B, np.float64)
    inv = [1.0 / (xs * ws1), 1.0 / (xs * ws2), 1.0 / (xs * ws3)]
    for r in res.results:
        o = r["o"].astype(np.float64)  # [M, 1024]
        for j in range(3):
            fr += o[2 * j] * inv[j]
            fi += o[2 * j + 1] * inv[j]
    psd = fr**2 + fi**2
    out = psd * float(np.asarray(fc_w).reshape(-1)[0]) + float(
        np.asarray(fc_b).reshape(-1)[0]
    )
    return out.astype(np.float32).reshape(B, 1), res


def kernel(input, freqs, fc_w, fc_b):
    out, _ = _run(input, freqs, fc_w, fc_b, trace=False)
    return out
